# revision 14
# baseline (speedup 1.0000x reference)
# Multi-head causal attention (B=2, T=2048, D=1024, H=16, HS=64) on 8 TRN2 NeuronCores.
#
# Sharding: core c = (batch b = c//4, head-group g = c%4 -> heads 4g..4g+3).
# Host pre-transposes x (kernel input xT = x[b].T) and slices w_qkv columns /
# w_out rows per core; each core computes a partial (T, D) output projection
# and the host sums the 4 partials per batch (+ b_out).
#
# On-device layout runs in "transposed activation" space:
#   Q^T,K^T [hs, t] come naturally out of the QKV projection (w stationary,
#   x^T moving); V is computed in natural [t, hs] layout (x^T stationary,
#   w_v moving) with an extra ones-column so the PV matmul produces both
#   o^T = V^T P^T and the softmax denominators l = 1^T P in one pass.
#   Scores are built as S^T [k, t] blocks (softmax needs no max-subtraction:
#   inputs are ~N(0,1), scores bounded, exp safe in fp32).
#   o^T [hs, t] then feeds the output projection as the stationary operand
#   with no further transposes.
import math
import os
import sys

import numpy as np

for _p in ("/opt/trn_rl_repo",):
    if _p not in sys.path and os.path.isdir(_p):
        sys.path.insert(0, _p)

import concourse.bass as bass
import concourse.mybir as mybir
import concourse.tile as tile
from concourse import bacc
from concourse import bass_utils

B, T, D = 2, 2048, 1024
H, HS = 16, 64
NCORES = 8
GROUPS = NCORES // B          # head-groups per batch = 4
HPC = H // GROUPS             # heads per core = 4
EC = HPC * HS                 # head-dim cols per section per core = 256
DC = D // 128                 # d-chunks = 8
TT = T // 128                 # t-tiles = 16
QS = 512                      # q-supertile
NQS = T // QS                 # 4
SCALE = 1.0 / math.sqrt(HS)

F32 = mybir.dt.float32
CDT = mybir.dt.bfloat16       # compute dtype for matmul operands


def _mha_tile_kernel(tc, outp, xT, wq, wo, bqk, bv, mask):
    nc = tc.nc
    EXP = mybir.ActivationFunctionType.Exp
    F32R = mybir.dt.float32r

    with (
        tc.tile_pool(name="singles", bufs=1) as singles,
        tc.tile_pool(name="acts", bufs=1) as acts,
        tc.tile_pool(name="pt", bufs=4) as ptp,
        tc.tile_pool(name="rl", bufs=4) as rlp,
        tc.tile_pool(name="ob", bufs=3) as obp,
        tc.tile_pool(name="psum", bufs=1, space="PSUM") as psa,
    ):
        # ---- input loads (cast fp32 -> CDT on SWDGE) ----
        xT_sb = singles.tile([128, DC, T], CDT)
        w_sb = singles.tile([128, DC, 3 * EC], CDT)
        xT_r = xT.rearrange("(c p) t -> p c t", p=128)
        wq_r = wq.rearrange("(c p) e -> p c e", p=128)
        for dc in range(DC):
            nc.gpsimd.dma_start(out=w_sb[:, dc, :], in_=wq_r[:, dc, :])
            nc.gpsimd.dma_start(out=xT_sb[:, dc, :], in_=xT_r[:, dc, :])
        bqk_sb = singles.tile([128, 2 * EC // 128], F32)
        nc.gpsimd.dma_start(out=bqk_sb, in_=bqk.rearrange("(c p) -> p c", p=128))
        bvb_sb = singles.tile([128, EC], F32)
        bv_b = bass.AP(tensor=bv.tensor, offset=bv.offset,
                       ap=[[0, 128]] + list(bv.ap))
        nc.gpsimd.dma_start(out=bvb_sb, in_=bv_b)
        mask_sb = singles.tile([128, 896], CDT)
        nc.gpsimd.dma_start(out=mask_sb, in_=mask)
        wo_sb = singles.tile([128, EC // 128, D], CDT)
        nc.gpsimd.dma_start(out=wo_sb, in_=wo.rearrange("(c p) e -> p c e", p=128))
        ones_f = singles.tile([1, 64], F32)
        nc.vector.memset(ones_f, 1.0)
        ones_sb = singles.tile([1, 64], F32R)
        nc.vector.tensor_copy(out=ones_sb, in_=ones_f)

        qkT_sb = acts.tile([128, 2 * EC // 128, T], CDT)
        vones_sb = acts.tile([128, TT, HPC, HS + 1], CDT)
        oT_sb = acts.tile([128, EC // 128, T], CDT)
        nc.vector.memset(vones_sb[:, :, :, HS:HS + 1], 1.0)

        # PSUM tags: "s" 2x[128,1024] (quads + QKV/V groups), "o" 2x[65,512]
        # (PV accumulators), "r" 2x[128,512] (1/l broadcast + out-proj halves).
        def emit_qk(et, ts):
            ps = psa.tile([128, 512], F32, tag="m", bufs=1, name="psqk")
            for dc in range(DC):
                nc.tensor.matmul(
                    ps,
                    lhsT=w_sb[:, dc, et * 128:(et + 1) * 128],
                    rhs=xT_sb[:, dc, ts * 512:(ts + 1) * 512],
                    start=(dc == 0),
                    stop=(dc == DC - 1),
                )
            nc.vector.tensor_scalar_add(
                out=qkT_sb[:, et, ts * 512:(ts + 1) * 512],
                in0=ps,
                scalar1=bqk_sb[:, et:et + 1],
            )

        def emit_v(tt):
            psv = psa.tile([128, EC], F32, tag="m", bufs=1, name="psv")
            for dc in range(DC):
                nc.tensor.matmul(
                    psv,
                    lhsT=xT_sb[:, dc, tt * 128:(tt + 1) * 128],
                    rhs=w_sb[:, dc, 2 * EC:3 * EC],
                    start=(dc == 0),
                    stop=(dc == DC - 1),
                )
            nc.vector.tensor_add(
                out=vones_sb[:, tt, :, 0:HS],
                in0=psv.rearrange("p (h s) -> p h s", h=HPC),
                in1=bvb_sb.rearrange("p (h s) -> p h s", h=HPC),
            )

        def attn_quads(qs):
            # quad = list of (kb, col_off, q0, nq); diagonal blocks packed
            # contiguously so one exp covers only valid columns.
            quads = []
            for kq in range(qs * 2):
                quads.append([(kq * 2, 0, 0, 512), (kq * 2 + 1, 512, 0, 512)])
            d0 = qs * 4
            quads.append([(d0 + 0, 0, 0, 512), (d0 + 1, 512, 128, 384)])
            quads.append([(d0 + 2, 0, 256, 256), (d0 + 3, 256, 384, 128)])
            return quads

        def emit_attn(h, qs):
            pb = 64 * (h % 2)
            qT = qkT_sb[pb:pb + 64, h // 2, :]
            kT = qkT_sb[pb:pb + 64, 2 + h // 2, :]
            po = psa.tile([65, 512], F32, tag="o", bufs=2)
            nblk = (qs + 1) * 4

            def emit_pv(pT, quad):
                for (kb, off, q0, nq) in quad:
                    nc.tensor.matmul(
                        po[:, q0:512],
                        lhsT=vones_sb[:, kb, h, :],
                        rhs=pT[:, off:off + nq],
                        start=(kb == 0),
                        stop=(kb == nblk - 1),
                    )

            prev = None
            for quad in attn_quads(qs):
                sps = psa.tile([128, 1024], F32, tag="s", bufs=2, name="sps")
                pT = ptp.tile([128, 1024], CDT, tag="pT", name="pT")
                for (kb, off, q0, nq) in quad:
                    nc.tensor.matmul(
                        sps[:, off:off + nq],
                        lhsT=kT[:, kb * 128:(kb + 1) * 128],
                        rhs=qT[:, qs * 512 + q0:(qs + 1) * 512],
                        start=True,
                        stop=True,
                    )
                w = max(off + nq for (kb, off, q0, nq) in quad)
                nc.scalar.activation(out=pT[:, 0:w], in_=sps[:, 0:w],
                                     func=EXP, scale=SCALE)
                for (kb, off, q0, nq) in quad:
                    if kb >= qs * 4:  # diagonal: mask leading 128-col triangle
                        nc.vector.tensor_mul(
                            out=pT[:, off:off + 128],
                            in0=pT[:, off:off + 128],
                            in1=mask_sb[:, 384:512],
                        )
                if prev is not None:
                    emit_pv(*prev)
                prev = (pT, quad)
            emit_pv(*prev)

            # epilogue: o^T = o^T_unnorm * (1/l), 1/l broadcast via K=1 matmul
            rl = rlp.tile([1, 512], F32R, tag="rl")
            with nc.allow_low_precision(reason="1/l broadcast via f32r matmul"):
                nc.vector.reciprocal(out=rl, in_=po[64:65, :])
            rlps = psa.tile([64, 512], F32, tag="r", bufs=1, name="rlps")
            nc.tensor.matmul(rlps, lhsT=ones_sb, rhs=rl, start=True, stop=True)
            rlb = rlp.tile([64, 512], F32, tag="rlb")
            nc.vector.tensor_copy(out=rlb, in_=rlps)
            nc.vector.tensor_mul(
                out=oT_sb[pb:pb + 64, h // 2, qs * 512:(qs + 1) * 512],
                in0=po[0:64, :],
                in1=rlb,
            )

        def emit_outproj(tt):
            outsb = obp.tile([128, 1024], F32, tag="ob", name="outsb")
            for half in range(2):
                pr = psa.tile([128, 512], F32, tag="r", bufs=1, name="pso")
                for ec in range(EC // 128):
                    nc.tensor.matmul(
                        pr,
                        lhsT=oT_sb[:, ec, tt * 128:(tt + 1) * 128],
                        rhs=wo_sb[:, ec, half * 512:(half + 1) * 512],
                        start=(ec == 0),
                        stop=(ec == EC // 128 - 1),
                    )
                if (tt + half) % 2 == 0:
                    nc.scalar.copy(out=outsb[:, half * 512:(half + 1) * 512], in_=pr)
                else:
                    nc.vector.tensor_copy(out=outsb[:, half * 512:(half + 1) * 512],
                                          in_=pr)
            nc.sync.dma_start(out=outp[tt * 128:(tt + 1) * 128, :], in_=outsb)

        # ---- interleaved emission: per q-round, feed ACT (exp) continuously;
        # next round's projections + previous round's out-proj are fillers
        # emitted between attention heads so PE gap-fills while ACT chews ----
        for et in (0, 2, 1, 3):
            emit_qk(et, 0)
        for tt in range(4):
            emit_v(tt)
        for qs in range(NQS):
            fillers = []
            if qs < NQS - 1:
                fillers += [lambda et=et: emit_qk(et, qs + 1) for et in (0, 2, 1, 3)]
                fillers += [lambda tt=tt: emit_v(tt) for tt in range(4 * qs + 4, 4 * qs + 8)]
            if qs >= 1:
                fillers += [lambda tt=tt: emit_outproj(tt) for tt in range(4 * (qs - 1), 4 * qs)]
            for h in range(HPC):
                emit_attn(h, qs)
                for f in fillers[(h * len(fillers)) // HPC:((h + 1) * len(fillers)) // HPC]:
                    f()
        for tt in range(4 * (NQS - 1), 4 * NQS):
            emit_outproj(tt)


def build_nc():
    nc = bacc.Bacc("TRN2", target_bir_lowering=False, debug=False)
    xT = nc.dram_tensor("xT", (D, T), F32, kind="ExternalInput")
    wq = nc.dram_tensor("wq", (D, 3 * EC), F32, kind="ExternalInput")
    wo = nc.dram_tensor("wo", (EC, D), F32, kind="ExternalInput")
    bqk = nc.dram_tensor("bqk", (2 * EC,), F32, kind="ExternalInput")
    bv = nc.dram_tensor("bv", (EC,), F32, kind="ExternalInput")
    mask = nc.dram_tensor("mask", (128, 896), CDT, kind="ExternalInput")
    outp = nc.dram_tensor("outp", (T, D), F32, kind="ExternalOutput")
    with tile.TileContext(nc) as tc:
        _mha_tile_kernel(tc, outp[:], xT[:], wq[:], wo[:], bqk[:], bv[:], mask[:])
    nc.compile()
    return nc


def host_mask():
    # big[x, j] = 1.0 where j >= x + 384 else 0  (bf16)
    import ml_dtypes
    x = np.arange(128)[:, None]
    j = np.arange(896)[None, :]
    return (j >= x + 384).astype(ml_dtypes.bfloat16)


def make_in_maps(x, w_qkv, b_qkv, w_out):
    mask = host_mask()
    in_maps = []
    for c in range(NCORES):
        b, g = divmod(c, GROUPS)
        cs = slice(EC * g, EC * (g + 1))
        wq_c = np.ascontiguousarray(
            np.concatenate(
                [w_qkv[:, cs], w_qkv[:, D:][:, cs], w_qkv[:, 2 * D:][:, cs]], axis=1
            )
        )
        in_maps.append({
            "xT": np.ascontiguousarray(x[b].T),
            "wq": wq_c,
            "wo": np.ascontiguousarray(w_out[cs, :]),
            "bqk": np.ascontiguousarray(
                np.concatenate([b_qkv[cs], b_qkv[D:][cs]])
            ),
            "bv": np.ascontiguousarray(b_qkv[2 * D:][cs]),
            "mask": mask,
        })
    return in_maps


_NC_CACHE = {}


def get_nc():
    if "nc" not in _NC_CACHE:
        _NC_CACHE["nc"] = build_nc()
    return _NC_CACHE["nc"]


def run_on_hw(in_maps, **kwargs):
    nc = get_nc()
    return bass_utils.run_bass_kernel_spmd(
        nc, in_maps, core_ids=list(range(NCORES)), **kwargs
    )


def kernel(x, w_qkv, b_qkv, w_out, b_out):
    x = np.asarray(x, dtype=np.float32)
    w_qkv = np.asarray(w_qkv, dtype=np.float32)
    b_qkv = np.asarray(b_qkv, dtype=np.float32)
    w_out = np.asarray(w_out, dtype=np.float32)
    b_out = np.asarray(b_out, dtype=np.float32)

    in_maps = make_in_maps(x, w_qkv, b_qkv, w_out)
    res = run_on_hw(in_maps)
    parts = [r["outp"].astype(np.float64) for r in res.results]
    out = np.stack([
        sum(parts[GROUPS * b:GROUPS * (b + 1)]) for b in range(B)
    ]).astype(np.float32)
    return out + b_out[None, None, :]
